# revision 13
# baseline (speedup 1.0000x reference)
"""Trainium2 Bass kernel for nn_DecoderOnlyTransformer_10041633538673.

Reference computation (B=2, S=2048, D=1024, L=1024, H=16, dh=64):
    q/k/v = split_heads(x @ Wq/Wk/Wv)           # [B*H, S, dh]
    scores[k, q] = <q_q, k_k> / sqrt(D)
    attn = softmax(scores, axis=q)              # quirk: softmax over QUERY axis
    out[q, v] = sum_k attn[k, q] * v[k, v]
    z = merge_heads(out) @ Wo
    z = l2norm(z); z = z @ Wff; z = l2norm(z); z = gelu(z)

Sharding over 8 cores (same Bass program on every core; all per-core
differences are carried in the input values):
    core c: batch b = c//4, rank r = c%4, owns heads 4r..4r+3 of batch b.
    - QKV projections + attention are head-sharded (softmax over q stays
      core-local; normalizer 1/Z folded into V rows, so no pass over E).
    - One 8-core AllToAll per head-pair re-shards from head-split to
      sequence-split: core j ends with stack[l, q] for its 512-query
      slice, l = all 2048 rows of [batch0 heads; batch1 heads].
    - Wo is fed per-core as wo2 = [Wo; 0] (batch 0 cores) or [0; Wo]
      (batch 1 cores) so the contraction over the stacked 2048 rows picks
      up exactly the same-batch contribution (rank-independent program).
      The Wo accumulation runs in two passes: pair-0 chunks start while
      pair-1's AllToAll is still in flight.
    - Tail (l2norm -> Wff -> l2norm -> gelu) runs on the local 512-row
      slice; host reassembles y[b, r*512:(r+1)*512, :] = out_core.

All matmuls run in bf16 (full double-pumped rate on TRN2).
"""

import os
import numpy as np

import concourse.bass as bass
import concourse.tile as tile
from concourse import bacc, mybir
from concourse.bass_utils import run_bass_kernel_spmd
from concourse.masks import make_identity

F32 = mybir.dt.float32
BF16 = mybir.dt.bfloat16

P = 128
S = 2048
D = 1024
NH = 4          # heads per core
DH = 64
LC = NH * DH    # 256 local head-cols per core
DC = D // P     # 8 contraction chunks
SBLK = S // P   # 16 seq blocks
SLICE = S // 4  # 512-query slice per core

AF = mybir.ActivationFunctionType
ALU = mybir.AluOpType

# CoreSim doesn't implement Gelu; flag swaps it for Identity in sim checks
SIM_NO_GELU = os.environ.get("KERNEL_SIM_NO_GELU", "0") == "1"


def build_program():
    nc = bacc.Bacc(
        "TRN2",
        target_bir_lowering=False,
        debug=False,
        enable_asserts=False,
        num_devices=8,
    )

    xT = nc.dram_tensor("xT", [D, S], BF16, kind="ExternalInput").ap()
    wq = nc.dram_tensor("wq", [D, LC], BF16, kind="ExternalInput").ap()
    wk = nc.dram_tensor("wk", [D, LC], BF16, kind="ExternalInput").ap()
    wv = nc.dram_tensor("wv", [D, LC], BF16, kind="ExternalInput").ap()
    wo2 = nc.dram_tensor("wo2", [2, 8 * P, D], BF16, kind="ExternalInput").ap()
    wff = nc.dram_tensor("wff", [D, D], BF16, kind="ExternalInput").ap()
    out = nc.dram_tensor("out", [SLICE, D], F32, kind="ExternalOutput").ap()

    cc_in = [
        nc.dram_tensor(f"cc_in{t}", [8 * P, SLICE], BF16).ap() for t in range(2)
    ]
    cc_out = [
        nc.dram_tensor(f"cc_out{t}", [8 * P, SLICE], BF16).ap() for t in range(2)
    ]

    with tile.TileContext(nc) as tc:
        misc = tc.alloc_tile_pool(name="misc", bufs=1)
        ident = misc.tile([P, P], F32)
        make_identity(nc, ident)

        qkv = tc.alloc_tile_pool(name="qkv", bufs=1)
        qt = [qkv.tile([P, S], BF16, tag=f"qt{t}", name=f"qt{t}") for t in range(2)]
        kt = [qkv.tile([P, S], BF16, tag=f"kt{t}", name=f"kt{t}") for t in range(2)]
        v_sb = qkv.tile([P, SBLK, LC], BF16, tag="v")

        ao_pool = tc.alloc_tile_pool(name="ao", bufs=1)
        ao = [ao_pool.tile([P, S], BF16, tag=f"ao{t}", name=f"ao{t}") for t in range(2)]

        # ---- Phase 1: load x^T + projection weights; compute Q^T, K^T, V
        with tc.tile_pool(name="xtw", bufs=1) as xtw, tc.tile_pool(
            name="pp1", bufs=1, space="PSUM"
        ) as pp1:
            wq_sb = xtw.tile([P, DC, LC], BF16, tag="wq")
            wk_sb = xtw.tile([P, DC, LC], BF16, tag="wk")
            wv_sb = xtw.tile([P, DC, LC], BF16, tag="wv")
            xt = xtw.tile([P, DC, S], BF16, tag="xt")
            # issue order matters: the first matmul wave needs only wq's
            # dc=0 chunk + xt chunk 0, so load those first and the rest
            # per-chunk behind them
            wq_r = wq.rearrange("(c p) m -> p c m", p=P)
            wk_r = wk.rearrange("(c p) m -> p c m", p=P)
            nc.sync.dma_start(out=wq_sb[:, 0, :], in_=wq_r[:, 0, :])
            nc.sync.dma_start(out=xt[:, 0, :], in_=xT[0:P, :])
            for dc in range(1, DC):
                nc.sync.dma_start(out=wq_sb[:, dc, :], in_=wq_r[:, dc, :])
            nc.sync.dma_start(out=wk_sb, in_=wk_r)
            for dc in range(1, DC):
                nc.sync.dma_start(out=xt[:, dc, :], in_=xT[dc * P : (dc + 1) * P, :])
            nc.sync.dma_start(out=wv_sb, in_=wv.rearrange("(c p) m -> p c m", p=P))

            # Q^T, K^T: [256 head-cols, S] as 2 tiles of [128, S].
            # dc-outer accumulation into 8 live PSUM tiles so the first
            # matmuls start as soon as xt chunk 0 lands.
            for w_sb, dst in ((wq_sb, qt), (wk_sb, kt)):
                pst = [
                    pp1.tile([P, 512], F32, tag=f"pj{i}", name=f"pj{i}")
                    for i in range(8)
                ]
                for dc in range(DC):
                    for i in range(8):
                        lb, sb = i // 4, i % 4
                        nc.tensor.matmul(
                            pst[i],
                            lhsT=w_sb[:, dc, lb * P : (lb + 1) * P],
                            rhs=xt[:, dc, sb * 512 : (sb + 1) * 512],
                            start=(dc == 0),
                            stop=(dc == DC - 1),
                        )
                for i in range(8):
                    lb, sb = i // 4, i % 4
                    nc.vector.tensor_copy(
                        out=dst[lb][:, sb * 512 : (sb + 1) * 512], in_=pst[i]
                    )
            # V natural: [S, 256] as [128, sblk, 256]
            for sb in range(SBLK):
                ps = pp1.tile([P, 512], F32, tag=f"pj{sb % 8}", name=f"pjv{sb}")
                for dc in range(DC):
                    nc.tensor.matmul(
                        ps[:, 0:LC],
                        lhsT=xt[:, dc, sb * P : (sb + 1) * P],
                        rhs=wv_sb[:, dc, :],
                        start=(dc == 0),
                        stop=(dc == DC - 1),
                    )
                nc.vector.tensor_copy(out=v_sb[:, sb, :], in_=ps[:, 0:LC])

        # weights for the post-attention phases (DMA overlaps attention)
        w2 = tc.alloc_tile_pool(name="w2", bufs=1)
        wo2_sb = w2.tile([P, 2, 8, D], BF16, tag="wo2")
        wff_sb = w2.tile([P, DC, D], BF16, tag="wff")
        nc.sync.dma_start(out=wo2_sb, in_=wo2.rearrange("t (i p) d -> p t i d", p=P))
        nc.sync.dma_start(out=wff_sb, in_=wff.rearrange("(c p) d -> p c d", p=P))

        # ---- Phase 2: attention, head-local. E = exp(scores/32); Z folded
        # into V rows; out^T accumulated per head in PSUM.
        with tc.tile_pool(name="att", bufs=4) as att, tc.tile_pool(
            name="sc", bufs=2, space="PSUM"
        ) as scp, tc.tile_pool(name="op", bufs=1, space="PSUM") as opp, tc.tile_pool(
            name="asml", bufs=4
        ) as asml:
            for t in range(2):
                o_pp = [
                    opp.tile([P, 512], F32, tag=f"op{qb}", name=f"op{qb}")
                    for qb in range(4)
                ]

                def av_block(kb, hh, zp, e_t):
                    # fold 1/Z into the V rows of this k-block, then
                    # accumulate out^T += vp^T @ E
                    zs = asml.tile([P, 1], F32, tag="zs", name="zs")
                    nc.vector.tensor_add(out=zs, in0=zp[:, 0:1], in1=zp[:, 1:2])
                    zr = asml.tile([P, 1], F32, tag="zr", name="zr")
                    nc.vector.reciprocal(out=zr, in_=zs)
                    vp = asml.tile([P, DH], BF16, tag="vp", name="vp")
                    nc.vector.tensor_scalar_mul(
                        out=vp,
                        in0=v_sb[:, kb, (2 * t + hh) * DH : (2 * t + hh + 1) * DH],
                        scalar1=zr,
                    )
                    for qb in range(4):
                        # head A -> PE column strip 0:64, head B -> 64:128
                        nc.tensor.matmul(
                            o_pp[qb][hh * DH : (hh + 1) * DH, :],
                            lhsT=vp,
                            rhs=e_t[:, qb * 512 : (qb + 1) * 512],
                            start=(kb == 0),
                            stop=(kb == SBLK - 1),
                            tile_position=(0, hh * DH),
                            skip_group_check=True,
                        )

                # software pipeline: AV of k-block kb-1 issues between the
                # score matmuls of kb, so the PE has work while exp frees
                # the score PSUM slots
                prev = None
                for kb in range(SBLK):
                    e_a = att.tile([P, S], BF16, tag="e", name="e_a")
                    e_b = att.tile([P, S], BF16, tag="e", name="e_b")
                    zp_a = asml.tile([P, 2], F32, tag="zp", name="zp_a")
                    zp_b = asml.tile([P, 2], F32, tag="zp", name="zp_b")
                    for qh in range(2):
                        sc_a = scp.tile([P, 1024], F32, tag="sc", name="sc_a")
                        sc_b = scp.tile([P, 1024], F32, tag="sc", name="sc_b")
                        # each head uses only 64 contract rows (dh) and each
                        # output half only 64 PE columns: the four 64x64
                        # quadrants are disjoint and co-execute on the PE
                        for qs in range(2):
                            q0 = qh * 1024 + qs * 512
                            for head, row0, dst in ((0, 0, sc_a), (1, DH, sc_b)):
                                for half in range(2):
                                    nc.tensor.matmul(
                                        dst[half * DH : (half + 1) * DH, qs * 512 : (qs + 1) * 512],
                                        lhsT=kt[t][row0 : row0 + DH, kb * P + half * DH : kb * P + (half + 1) * DH],
                                        rhs=qt[t][row0 : row0 + DH, q0 : q0 + 512],
                                        start=True,
                                        stop=True,
                                        tile_position=(row0, half * DH),
                                        skip_group_check=True,
                                    )
                        nc.scalar.activation(
                            out=e_a[:, qh * 1024 : (qh + 1) * 1024],
                            in_=sc_a,
                            func=AF.Exp,
                            scale=1.0 / 32.0,
                            accum_out=zp_a[:, qh : qh + 1],
                        )
                        nc.scalar.activation(
                            out=e_b[:, qh * 1024 : (qh + 1) * 1024],
                            in_=sc_b,
                            func=AF.Exp,
                            scale=1.0 / 32.0,
                            accum_out=zp_b[:, qh : qh + 1],
                        )
                    if prev is not None:
                        av_block(kb - 1, 0, prev[0], prev[2])
                        av_block(kb - 1, 1, prev[1], prev[3])
                    prev = (zp_a, zp_b, e_a, e_b)
                av_block(SBLK - 1, 0, prev[0], prev[2])
                av_block(SBLK - 1, 1, prev[1], prev[3])
                # pair t done: ship its AllToAll now so it hides under the
                # next pair's attention compute. DMA each query block as
                # soon as its PSUM->SBUF copy lands.
                for qb in range(4):
                    nc.vector.tensor_copy(
                        out=ao[t][:, qb * 512 : (qb + 1) * 512],
                        in_=o_pp[qb],
                    )
                    for j in (qb, qb + 4):
                        nc.sync.dma_start(
                            out=cc_in[t][j * P : (j + 1) * P, :],
                            in_=ao[t][:, qb * 512 : (qb + 1) * 512],
                        )
                nc.gpsimd.collective_compute(
                    "AllToAll",
                    ALU.bypass,
                    replica_groups=[[0, 1, 2, 3, 4, 5, 6, 7]],
                    ins=[cc_in[t]],
                    outs=[cc_out[t]],
                )

        # ---- Phase 4/5: Wo (batch-masked), l2norm, Wff, l2norm, gelu
        with tc.tile_pool(name="tail", bufs=2) as tl, tc.tile_pool(
            name="tsm", bufs=1
        ) as tsm:
            st = w2.tile([P, 2, 8, SLICE], BF16, tag="st")
            for t in range(2):
                for i in range(8):
                    nc.sync.dma_start(
                        out=st[:, t, i, :], in_=cc_out[t][i * P : (i + 1) * P, :]
                    )

            z_sb = tl.tile([P, 4, D], F32, tag="big")
            # Wo in two passes over pairs: pair-0 chunks accumulate into 8
            # live PSUM banks while pair-1's AllToAll is still in flight.
            with tc.tile_pool(name="ppwo", bufs=1, space="PSUM") as ppwo:
                wo_pp = [
                    ppwo.tile([P, 512], F32, tag=f"wo{sb}{db}", name=f"wo{sb}{db}")
                    for sb in range(4)
                    for db in range(2)
                ]
                for t in range(2):
                    for sb in range(4):
                        for db in range(2):
                            for i in range(8):
                                nc.tensor.matmul(
                                    wo_pp[sb * 2 + db],
                                    lhsT=st[:, t, i, sb * P : (sb + 1) * P],
                                    rhs=wo2_sb[:, t, i, db * 512 : (db + 1) * 512],
                                    start=(t == 0 and i == 0),
                                    stop=(t == 1 and i == 7),
                                )
                junk = tsm.tile([P, D], F32, tag="junk")
                ss1h = tsm.tile([P, 8], F32, tag="ss1h")
                for sb in range(4):
                    for db in range(2):
                        # square-accumulate straight from PSUM on the ACT
                        # engine while the DVE copies the same tile to SBUF
                        nc.scalar.activation(
                            out=junk[:, 0:512],
                            in_=wo_pp[sb * 2 + db],
                            func=AF.Square,
                            accum_out=ss1h[:, sb * 2 + db : sb * 2 + db + 1],
                        )
                        nc.vector.tensor_copy(
                            out=z_sb[:, sb, db * 512 : (db + 1) * 512],
                            in_=wo_pp[sb * 2 + db],
                        )

            ss1 = tsm.tile([P, 4], F32, tag="ss1")
            rs1 = tsm.tile([P, 4], F32, tag="rs1")
            for sb in range(4):
                nc.vector.tensor_add(
                    out=ss1[:, sb : sb + 1],
                    in0=ss1h[:, 2 * sb : 2 * sb + 1],
                    in1=ss1h[:, 2 * sb + 1 : 2 * sb + 2],
                )
            # 1/sqrt(ss) = exp(-0.5 * ln(ss)); Ln+Exp share one ACT table set
            nc.scalar.activation(out=ss1, in_=ss1, func=AF.Ln)
            nc.scalar.activation(out=rs1, in_=ss1, func=AF.Exp, scale=-0.5)
            for sb in range(4):
                nc.vector.tensor_scalar_mul(
                    out=z_sb[:, sb, :], in0=z_sb[:, sb, :], scalar1=rs1[:, sb : sb + 1]
                )

            # n1^T via PE transpose, software-pipelined with the Wff
            # matmuls: transpose block sb+1 issues before Wff block sb
            with tc.tile_pool(name="pp2", bufs=4, space="PSUM") as pp2:
                n1t = tsm.tile([P, DC, SLICE], BF16, tag="n1t")
                y_sb = tl.tile([P, 4, D], F32, tag="big")
                ss2 = tsm.tile([P, 4], F32, tag="ss2")
                rs2 = tsm.tile([P, 4], F32, tag="rs2")

                def tr_block(sb):
                    for dc in range(DC):
                        tp = pp2.tile([P, P], F32, tag="tp")
                        nc.tensor.transpose(
                            tp, z_sb[:, sb, dc * P : (dc + 1) * P], ident
                        )
                        nc.vector.tensor_copy(
                            out=n1t[:, dc, sb * P : (sb + 1) * P], in_=tp
                        )

                ss2h = tsm.tile([P, 8], F32, tag="ss2h")

                def ff_block(sb):
                    for db in range(2):
                        ps = pp2.tile([P, 512], F32, tag="pp2")
                        for dc in range(DC):
                            nc.tensor.matmul(
                                ps,
                                lhsT=n1t[:, dc, sb * P : (sb + 1) * P],
                                rhs=wff_sb[:, dc, db * 512 : (db + 1) * 512],
                                start=(dc == 0),
                                stop=(dc == DC - 1),
                            )
                        # square-accumulate straight from PSUM while the
                        # DVE copies the same tile to SBUF
                        nc.scalar.activation(
                            out=junk[:, 0:512],
                            in_=ps,
                            func=AF.Square,
                            accum_out=ss2h[:, sb * 2 + db : sb * 2 + db + 1],
                        )
                        nc.vector.tensor_copy(
                            out=y_sb[:, sb, db * 512 : (db + 1) * 512], in_=ps
                        )

                tr_block(0)
                tr_block(1)
                ff_block(0)
                tr_block(2)
                ff_block(1)
                tr_block(3)
                ff_block(2)
                ff_block(3)
                for sb in range(4):
                    nc.vector.tensor_add(
                        out=ss2[:, sb : sb + 1],
                        in0=ss2h[:, 2 * sb : 2 * sb + 1],
                        in1=ss2h[:, 2 * sb + 1 : 2 * sb + 2],
                    )
                nc.scalar.activation(out=ss2, in_=ss2, func=AF.Ln)
                nc.scalar.activation(out=rs2, in_=ss2, func=AF.Exp, scale=-0.5)

                o_sb = tl.tile([P, 4, D], F32, tag="big")
                out_r = out.rearrange("(sb p) d -> p sb d", p=P)
                for sb in range(4):
                    nc.scalar.activation(
                        out=o_sb[:, sb, :],
                        in_=y_sb[:, sb, :],
                        func=AF.Identity if SIM_NO_GELU else AF.Gelu,
                        scale=rs2[:, sb : sb + 1],
                    )
                    nc.sync.dma_start(out=out_r[:, sb, :], in_=o_sb[:, sb, :])

        w2.release()
        ao_pool.release()
        qkv.release()
        misc.release()

    nc.compile()
    return nc


_NC = None


def _get_nc():
    global _NC
    if _NC is None:
        _NC = build_program()
    return _NC


def make_in_maps(x, Wq, Wk, Wv, Wo, Wff):
    import ml_dtypes

    bf = np.dtype(ml_dtypes.bfloat16)
    # wo2[t, i*128+p, :] = Wo[(i%4)*256 + t*128 + p, :] if core i is in my
    # batch group else 0  (matches the AllToAll stack row order per pair t)
    wo2_b = []
    for b in range(2):
        w = np.zeros((2, 8 * 128, 1024), dtype=np.float32)
        for t in range(2):
            for i in range(8):
                if i // 4 == b:
                    r0 = (i % 4) * 256 + t * 128
                    w[t, i * 128 : (i + 1) * 128, :] = Wo[r0 : r0 + 128, :]
        wo2_b.append(np.ascontiguousarray(w).astype(bf))
    wff_c = np.ascontiguousarray(Wff).astype(bf)
    in_maps = []
    for c in range(8):
        b, r = c // 4, c % 4
        in_maps.append(
            {
                "xT": np.ascontiguousarray(x[b].T).astype(bf),
                "wq": np.ascontiguousarray(Wq[:, r * LC : (r + 1) * LC]).astype(bf),
                "wk": np.ascontiguousarray(Wk[:, r * LC : (r + 1) * LC]).astype(bf),
                "wv": np.ascontiguousarray(Wv[:, r * LC : (r + 1) * LC]).astype(bf),
                "wo2": wo2_b[b],
                "wff": wff_c,
            }
        )
    return in_maps


def run(x, Wq, Wk, Wv, Wo, Wff, trace=False, **spmd_kwargs):
    nc = _get_nc()
    in_maps = make_in_maps(x, Wq, Wk, Wv, Wo, Wff)
    res = run_bass_kernel_spmd(
        nc, in_maps, core_ids=list(range(8)), trace=trace, **spmd_kwargs
    )
    y = np.empty((2, S, D), dtype=np.float32)
    for c in range(8):
        b, r = c // 4, c % 4
        y[b, r * SLICE : (r + 1) * SLICE, :] = res.results[c]["out"]
    return y, res


def kernel(x, Wq, Wk, Wv, Wo, Wff):
    y, _ = run(x, Wq, Wk, Wv, Wo, Wff)
    return y


# revision 14
# speedup vs baseline: 1.1697x; 1.1697x over previous
"""Trainium2 Bass kernel for nn_DecoderOnlyTransformer_10041633538673.

Reference computation (B=2, S=2048, D=1024, L=1024, H=16, dh=64):
    q/k/v = split_heads(x @ Wq/Wk/Wv)           # [B*H, S, dh]
    scores[k, q] = <q_q, k_k> / sqrt(D)
    attn = softmax(scores, axis=q)              # quirk: softmax over QUERY axis
    out[q, v] = sum_k attn[k, q] * v[k, v]
    z = merge_heads(out) @ Wo
    z = l2norm(z); z = z @ Wff; z = l2norm(z); z = gelu(z)

Sharding over 8 cores (same Bass program on every core; all per-core
differences are carried in the input values):
    core c: batch b = c//4, rank r = c%4, owns heads 4r..4r+3 of batch b.
    - QKV projections + attention are head-sharded (softmax over q stays
      core-local; normalizer 1/Z folded into V rows, so no pass over E).
    - One 8-core AllToAll per head-pair re-shards from head-split to
      sequence-split: core j ends with stack[l, q] for its 512-query
      slice, l = all 2048 rows of [batch0 heads; batch1 heads].
    - Wo is fed per-core as wo2 = [Wo; 0] (batch 0 cores) or [0; Wo]
      (batch 1 cores) so the contraction over the stacked 2048 rows picks
      up exactly the same-batch contribution (rank-independent program).
      The Wo accumulation runs in two passes: pair-0 chunks start while
      pair-1's AllToAll is still in flight.
    - Tail (l2norm -> Wff -> l2norm -> gelu) runs on the local 512-row
      slice; host reassembles y[b, r*512:(r+1)*512, :] = out_core.

All matmuls run in bf16 (full double-pumped rate on TRN2).
"""

import os
import numpy as np

import concourse.bass as bass
import concourse.tile as tile
from concourse import bacc, mybir
from concourse.bass_utils import run_bass_kernel_spmd
from concourse.masks import make_identity

F32 = mybir.dt.float32
BF16 = mybir.dt.bfloat16

P = 128
S = 2048
D = 1024
NH = 4          # heads per core
DH = 64
LC = NH * DH    # 256 local head-cols per core
DC = D // P     # 8 contraction chunks
SBLK = S // P   # 16 seq blocks
SLICE = S // 4  # 512-query slice per core

AF = mybir.ActivationFunctionType
ALU = mybir.AluOpType

# CoreSim doesn't implement Gelu; flag swaps it for Identity in sim checks
SIM_NO_GELU = os.environ.get("KERNEL_SIM_NO_GELU", "0") == "1"


def build_program():
    nc = bacc.Bacc(
        "TRN2",
        target_bir_lowering=False,
        debug=False,
        enable_asserts=False,
        num_devices=8,
    )

    xT = nc.dram_tensor("xT", [D, S], BF16, kind="ExternalInput").ap()
    wq = nc.dram_tensor("wq", [D, LC], BF16, kind="ExternalInput").ap()
    wk = nc.dram_tensor("wk", [D, LC], BF16, kind="ExternalInput").ap()
    wv = nc.dram_tensor("wv", [D, LC], BF16, kind="ExternalInput").ap()
    wo2 = nc.dram_tensor("wo2", [2, 8 * P, D], BF16, kind="ExternalInput").ap()
    wff = nc.dram_tensor("wff", [D, D], BF16, kind="ExternalInput").ap()
    out = nc.dram_tensor("out", [SLICE, D], F32, kind="ExternalOutput").ap()

    cc_in = [
        nc.dram_tensor(f"cc_in{t}", [8 * P, SLICE], BF16).ap() for t in range(2)
    ]
    cc_out = [
        nc.dram_tensor(f"cc_out{t}", [8 * P, SLICE], BF16).ap() for t in range(2)
    ]

    with tile.TileContext(nc) as tc:
        misc = tc.alloc_tile_pool(name="misc", bufs=1)
        ident = misc.tile([P, P], F32)
        make_identity(nc, ident)

        qkv = tc.alloc_tile_pool(name="qkv", bufs=1)
        qt = [qkv.tile([P, S], BF16, tag=f"qt{t}", name=f"qt{t}") for t in range(2)]
        kt = [qkv.tile([P, S], BF16, tag=f"kt{t}", name=f"kt{t}") for t in range(2)]
        v_sb = qkv.tile([P, SBLK, LC], BF16, tag="v")

        ao_pool = tc.alloc_tile_pool(name="ao", bufs=1)
        ao = [ao_pool.tile([P, S], BF16, tag=f"ao{t}", name=f"ao{t}") for t in range(2)]

        # ---- Phase 1: load x^T + projection weights; compute Q^T, K^T, V
        with tc.tile_pool(name="xtw", bufs=1) as xtw, tc.tile_pool(
            name="pp1", bufs=1, space="PSUM"
        ) as pp1:
            wq_sb = xtw.tile([P, DC, LC], BF16, tag="wq")
            wk_sb = xtw.tile([P, DC, LC], BF16, tag="wk")
            wv_sb = xtw.tile([P, DC, LC], BF16, tag="wv")
            xt = xtw.tile([P, DC, S], BF16, tag="xt")
            # issue order matters: the first matmul wave needs only wq's
            # dc=0 chunk + xt chunk 0, so load those first and the rest
            # per-chunk behind them
            wq_r = wq.rearrange("(c p) m -> p c m", p=P)
            wk_r = wk.rearrange("(c p) m -> p c m", p=P)
            nc.sync.dma_start(out=wq_sb[:, 0, :], in_=wq_r[:, 0, :])
            nc.sync.dma_start(out=xt[:, 0, :], in_=xT[0:P, :])
            for dc in range(1, DC):
                nc.sync.dma_start(out=wq_sb[:, dc, :], in_=wq_r[:, dc, :])
            nc.sync.dma_start(out=wk_sb, in_=wk_r)
            for dc in range(1, DC):
                nc.sync.dma_start(out=xt[:, dc, :], in_=xT[dc * P : (dc + 1) * P, :])
            nc.sync.dma_start(out=wv_sb, in_=wv.rearrange("(c p) m -> p c m", p=P))

            # Q^T, K^T: [256 head-cols, S] as 2 tiles of [128, S].
            # dc-outer accumulation into 8 live PSUM tiles so the first
            # matmuls start as soon as xt chunk 0 lands.
            for w_sb, dst in ((wq_sb, qt), (wk_sb, kt)):
                pst = [
                    pp1.tile([P, 512], F32, tag=f"pj{i}", name=f"pj{i}")
                    for i in range(8)
                ]
                for dc in range(DC):
                    for i in range(8):
                        lb, sb = i // 4, i % 4
                        nc.tensor.matmul(
                            pst[i],
                            lhsT=w_sb[:, dc, lb * P : (lb + 1) * P],
                            rhs=xt[:, dc, sb * 512 : (sb + 1) * 512],
                            start=(dc == 0),
                            stop=(dc == DC - 1),
                        )
                for i in range(8):
                    lb, sb = i // 4, i % 4
                    nc.vector.tensor_copy(
                        out=dst[lb][:, sb * 512 : (sb + 1) * 512], in_=pst[i]
                    )
            # V natural: [S, 256] as [128, sblk, 256]
            for sb in range(SBLK):
                ps = pp1.tile([P, 512], F32, tag=f"pj{sb % 8}", name=f"pjv{sb}")
                for dc in range(DC):
                    nc.tensor.matmul(
                        ps[:, 0:LC],
                        lhsT=xt[:, dc, sb * P : (sb + 1) * P],
                        rhs=wv_sb[:, dc, :],
                        start=(dc == 0),
                        stop=(dc == DC - 1),
                    )
                nc.vector.tensor_copy(out=v_sb[:, sb, :], in_=ps[:, 0:LC])

        # weights for the post-attention phases (DMA overlaps attention)
        w2 = tc.alloc_tile_pool(name="w2", bufs=1)
        wo2_sb = w2.tile([P, 2, 8, D], BF16, tag="wo2")
        wff_sb = w2.tile([P, DC, D], BF16, tag="wff")
        nc.sync.dma_start(out=wo2_sb, in_=wo2.rearrange("t (i p) d -> p t i d", p=P))
        nc.sync.dma_start(out=wff_sb, in_=wff.rearrange("(c p) d -> p c d", p=P))

        # ---- Phase 2: attention, head-local. E = exp(scores/32); Z folded
        # into V rows; out^T accumulated per head in PSUM.
        with tc.tile_pool(name="att", bufs=4) as att, tc.tile_pool(
            name="sc", bufs=2, space="PSUM"
        ) as scp, tc.tile_pool(name="op", bufs=1, space="PSUM") as opp, tc.tile_pool(
            name="asml", bufs=4
        ) as asml:
            for t in range(2):
                o_pp = [
                    opp.tile([P, 512], F32, tag=f"op{qb}", name=f"op{qb}")
                    for qb in range(4)
                ]

                def av_block(kb, hh, zp, e_t):
                    # fold 1/Z into the V rows of this k-block, then
                    # accumulate out^T += vp^T @ E
                    zs = asml.tile([P, 1], F32, tag="zs", name="zs")
                    nc.vector.tensor_add(out=zs, in0=zp[:, 0:1], in1=zp[:, 1:2])
                    zr = asml.tile([P, 1], F32, tag="zr", name="zr")
                    nc.vector.reciprocal(out=zr, in_=zs)
                    vp = asml.tile([P, DH], BF16, tag="vp", name="vp")
                    nc.vector.tensor_scalar_mul(
                        out=vp,
                        in0=v_sb[:, kb, (2 * t + hh) * DH : (2 * t + hh + 1) * DH],
                        scalar1=zr,
                    )
                    for qb in range(4):
                        # head A -> PE column strip 0:64, head B -> 64:128
                        nc.tensor.matmul(
                            o_pp[qb][hh * DH : (hh + 1) * DH, :],
                            lhsT=vp,
                            rhs=e_t[:, qb * 512 : (qb + 1) * 512],
                            start=(kb == 0),
                            stop=(kb == SBLK - 1),
                            tile_position=(0, hh * DH),
                            skip_group_check=True,
                        )

                # software pipeline: AV of k-block kb-1 issues between the
                # score matmuls of kb, so the PE has work while exp frees
                # the score PSUM slots
                prev = None
                for kb in range(SBLK):
                    e_a = att.tile([P, S], BF16, tag="e", name="e_a")
                    e_b = att.tile([P, S], BF16, tag="e", name="e_b")
                    zp_a = asml.tile([P, 2], F32, tag="zp", name="zp_a")
                    zp_b = asml.tile([P, 2], F32, tag="zp", name="zp_b")
                    for qh in range(2):
                        sc_a = scp.tile([P, 1024], F32, tag="sc", name="sc_a")
                        sc_b = scp.tile([P, 1024], F32, tag="sc", name="sc_b")
                        # each head uses only 64 contract rows (dh) and each
                        # output half only 64 PE columns: the four 64x64
                        # quadrants are disjoint and co-execute on the PE
                        for qs in range(2):
                            q0 = qh * 1024 + qs * 512
                            for head, row0, dst in ((0, 0, sc_a), (1, DH, sc_b)):
                                for half in range(2):
                                    nc.tensor.matmul(
                                        dst[half * DH : (half + 1) * DH, qs * 512 : (qs + 1) * 512],
                                        lhsT=kt[t][row0 : row0 + DH, kb * P + half * DH : kb * P + (half + 1) * DH],
                                        rhs=qt[t][row0 : row0 + DH, q0 : q0 + 512],
                                        start=True,
                                        stop=True,
                                        tile_position=(row0, half * DH),
                                        skip_group_check=True,
                                    )
                        nc.scalar.activation(
                            out=e_a[:, qh * 1024 : (qh + 1) * 1024],
                            in_=sc_a,
                            func=AF.Exp,
                            scale=1.0 / 32.0,
                            accum_out=zp_a[:, qh : qh + 1],
                        )
                        nc.scalar.activation(
                            out=e_b[:, qh * 1024 : (qh + 1) * 1024],
                            in_=sc_b,
                            func=AF.Exp,
                            scale=1.0 / 32.0,
                            accum_out=zp_b[:, qh : qh + 1],
                        )
                    if prev is not None:
                        av_block(kb - 1, 0, prev[0], prev[2])
                        av_block(kb - 1, 1, prev[1], prev[3])
                    prev = (zp_a, zp_b, e_a, e_b)
                av_block(SBLK - 1, 0, prev[0], prev[2])
                av_block(SBLK - 1, 1, prev[1], prev[3])
                # pair t done: ship its AllToAll now so it hides under the
                # next pair's attention compute. DMA each query block as
                # soon as its PSUM->SBUF copy lands.
                for qb in range(4):
                    nc.vector.tensor_copy(
                        out=ao[t][:, qb * 512 : (qb + 1) * 512],
                        in_=o_pp[qb],
                    )
                    for j in (qb, qb + 4):
                        nc.sync.dma_start(
                            out=cc_in[t][j * P : (j + 1) * P, :],
                            in_=ao[t][:, qb * 512 : (qb + 1) * 512],
                        )
                nc.gpsimd.collective_compute(
                    "AllToAll",
                    ALU.bypass,
                    replica_groups=[[0, 1, 2, 3, 4, 5, 6, 7]],
                    ins=[cc_in[t]],
                    outs=[cc_out[t]],
                )

        # ---- Phase 4/5: Wo (batch-masked), l2norm, Wff, l2norm, gelu
        with tc.tile_pool(name="tail", bufs=2) as tl, tc.tile_pool(
            name="tsm", bufs=1
        ) as tsm:
            st = w2.tile([P, 2, 8, SLICE], BF16, tag="st")
            for t in range(2):
                for i in range(8):
                    nc.sync.dma_start(
                        out=st[:, t, i, :], in_=cc_out[t][i * P : (i + 1) * P, :]
                    )

            z_sb = tl.tile([P, 4, D], F32, tag="big")
            # Wo in two passes over pairs: pair-0 chunks accumulate into 8
            # live PSUM banks while pair-1's AllToAll is still in flight.
            with tc.tile_pool(name="ppwo", bufs=1, space="PSUM") as ppwo:
                wo_pp = [
                    ppwo.tile([P, 512], F32, tag=f"wo{sb}{db}", name=f"wo{sb}{db}")
                    for sb in range(4)
                    for db in range(2)
                ]
                for t in range(2):
                    for sb in range(4):
                        for db in range(2):
                            for i in range(8):
                                nc.tensor.matmul(
                                    wo_pp[sb * 2 + db],
                                    lhsT=st[:, t, i, sb * P : (sb + 1) * P],
                                    rhs=wo2_sb[:, t, i, db * 512 : (db + 1) * 512],
                                    start=(t == 0 and i == 0),
                                    stop=(t == 1 and i == 7),
                                )
                junk = tsm.tile([P, D], F32, tag="junk")
                ss1h = tsm.tile([P, 8], F32, tag="ss1h")
                for sb in range(4):
                    for db in range(2):
                        # square-accumulate straight from PSUM on the ACT
                        # engine while the DVE copies the same tile to SBUF
                        nc.scalar.activation(
                            out=junk[:, 0:512],
                            in_=wo_pp[sb * 2 + db],
                            func=AF.Square,
                            accum_out=ss1h[:, sb * 2 + db : sb * 2 + db + 1],
                        )
                        nc.vector.tensor_copy(
                            out=z_sb[:, sb, db * 512 : (db + 1) * 512],
                            in_=wo_pp[sb * 2 + db],
                        )

            ss1 = tsm.tile([P, 4], F32, tag="ss1")
            rs1 = tsm.tile([P, 4], F32, tag="rs1")
            for sb in range(4):
                nc.vector.tensor_add(
                    out=ss1[:, sb : sb + 1],
                    in0=ss1h[:, 2 * sb : 2 * sb + 1],
                    in1=ss1h[:, 2 * sb + 1 : 2 * sb + 2],
                )
            # 1/sqrt(ss) = exp(-0.5 * ln(ss)); Ln+Exp share one ACT table set
            nc.scalar.activation(out=ss1, in_=ss1, func=AF.Ln)
            nc.scalar.activation(out=rs1, in_=ss1, func=AF.Exp, scale=-0.5)
            for sb in range(4):
                nc.vector.tensor_scalar_mul(
                    out=z_sb[:, sb, :], in0=z_sb[:, sb, :], scalar1=rs1[:, sb : sb + 1]
                )

            # n1^T via PE transpose, software-pipelined with the Wff
            # matmuls: transpose block sb+1 issues before Wff block sb
            with tc.tile_pool(name="pp2", bufs=4, space="PSUM") as pp2:
                n1t = tsm.tile([P, DC, SLICE], BF16, tag="n1t")
                y_sb = tl.tile([P, 4, D], F32, tag="big")
                ss2 = tsm.tile([P, 4], F32, tag="ss2")
                rs2 = tsm.tile([P, 4], F32, tag="rs2")

                def tr_block(sb):
                    for dc in range(DC):
                        tp = pp2.tile([P, P], F32, tag="tp")
                        nc.tensor.transpose(
                            tp, z_sb[:, sb, dc * P : (dc + 1) * P], ident
                        )
                        nc.vector.tensor_copy(
                            out=n1t[:, dc, sb * P : (sb + 1) * P], in_=tp
                        )

                def ff_block(sb):
                    for db in range(2):
                        ps = pp2.tile([P, 512], F32, tag="pp2")
                        for dc in range(DC):
                            nc.tensor.matmul(
                                ps,
                                lhsT=n1t[:, dc, sb * P : (sb + 1) * P],
                                rhs=wff_sb[:, dc, db * 512 : (db + 1) * 512],
                                start=(dc == 0),
                                stop=(dc == DC - 1),
                            )
                        nc.vector.tensor_copy(
                            out=y_sb[:, sb, db * 512 : (db + 1) * 512], in_=ps
                        )
                    nc.scalar.activation(
                        out=junk,
                        in_=y_sb[:, sb, :],
                        func=AF.Square,
                        accum_out=ss2[:, sb : sb + 1],
                    )

                tr_block(0)
                tr_block(1)
                ff_block(0)
                tr_block(2)
                ff_block(1)
                tr_block(3)
                ff_block(2)
                ff_block(3)
                nc.scalar.activation(out=ss2, in_=ss2, func=AF.Ln)
                nc.scalar.activation(out=rs2, in_=ss2, func=AF.Exp, scale=-0.5)

                o_sb = tl.tile([P, 4, D], F32, tag="big")
                out_r = out.rearrange("(sb p) d -> p sb d", p=P)
                for sb in range(4):
                    nc.scalar.activation(
                        out=o_sb[:, sb, :],
                        in_=y_sb[:, sb, :],
                        func=AF.Identity if SIM_NO_GELU else AF.Gelu,
                        scale=rs2[:, sb : sb + 1],
                    )
                    nc.sync.dma_start(out=out_r[:, sb, :], in_=o_sb[:, sb, :])

        w2.release()
        ao_pool.release()
        qkv.release()
        misc.release()

    nc.compile()
    return nc


_NC = None


def _get_nc():
    global _NC
    if _NC is None:
        _NC = build_program()
    return _NC


def make_in_maps(x, Wq, Wk, Wv, Wo, Wff):
    import ml_dtypes

    bf = np.dtype(ml_dtypes.bfloat16)
    # wo2[t, i*128+p, :] = Wo[(i%4)*256 + t*128 + p, :] if core i is in my
    # batch group else 0  (matches the AllToAll stack row order per pair t)
    wo2_b = []
    for b in range(2):
        w = np.zeros((2, 8 * 128, 1024), dtype=np.float32)
        for t in range(2):
            for i in range(8):
                if i // 4 == b:
                    r0 = (i % 4) * 256 + t * 128
                    w[t, i * 128 : (i + 1) * 128, :] = Wo[r0 : r0 + 128, :]
        wo2_b.append(np.ascontiguousarray(w).astype(bf))
    wff_c = np.ascontiguousarray(Wff).astype(bf)
    in_maps = []
    for c in range(8):
        b, r = c // 4, c % 4
        in_maps.append(
            {
                "xT": np.ascontiguousarray(x[b].T).astype(bf),
                "wq": np.ascontiguousarray(Wq[:, r * LC : (r + 1) * LC]).astype(bf),
                "wk": np.ascontiguousarray(Wk[:, r * LC : (r + 1) * LC]).astype(bf),
                "wv": np.ascontiguousarray(Wv[:, r * LC : (r + 1) * LC]).astype(bf),
                "wo2": wo2_b[b],
                "wff": wff_c,
            }
        )
    return in_maps


def run(x, Wq, Wk, Wv, Wo, Wff, trace=False, **spmd_kwargs):
    nc = _get_nc()
    in_maps = make_in_maps(x, Wq, Wk, Wv, Wo, Wff)
    res = run_bass_kernel_spmd(
        nc, in_maps, core_ids=list(range(8)), trace=trace, **spmd_kwargs
    )
    y = np.empty((2, S, D), dtype=np.float32)
    for c in range(8):
        b, r = c // 4, c % 4
        y[b, r * SLICE : (r + 1) * SLICE, :] = res.results[c]["out"]
    return y, res


def kernel(x, Wq, Wk, Wv, Wo, Wff):
    y, _ = run(x, Wq, Wk, Wv, Wo, Wff)
    return y
